# revision 51
# baseline (speedup 1.0000x reference)
"""Butterworth 4th-order lowpass (2 cascaded biquads) on 8 TRN2 NeuronCores.

Algorithm: block state-space decomposition of the IIR cascade.
  - Chunk the time axis into L=128 blocks. Within a chunk, the zero-state
    response is a lower-triangular Toeplitz matmul; chunk-boundary states
    follow s_k = M s_{k-1} + f_k with M = A^L, diagonalized into two
    complex modes solved by first-order REAL scans (DVE tensor_tensor_scan)
    via the rotation trick.
  - The output is computed directly in chunk-row-major layout as
       Y[col, t] = (X^T)^T H^T + S^T G''^T
    i.e. matmul with lhsT = time-major X slab (bf16) and rhs = H^T (bf16),
    plus an accumulated state-correction matmul (lhsT = S, rhs = G''^T).
    bf16 matmuls run 1 cycle/row even at 128-wide outputs, and the result
    lands store-ready (no output transpose pass).
  - The state path (F-pass projection, scan) stays f32 for accuracy.
Sharding: 256 independent signals, 32 per core, no cross-core comm.
"""
import numpy as np
from contextlib import ExitStack

import concourse.bass as bass
import concourse.tile as tile
from concourse import bacc, mybir
from concourse.bass_utils import run_bass_kernel_spmd

dt = mybir.dt

B, C, T_FULL = 32, 8, 96000
N_CORES = 8
NSIG = (B * C) // N_CORES      # 32 signals per core
L = 128                        # chunk length
SEG = (8, 8, 8, 8)             # segment sizes; 4*size and 4*offset must be 32-aligned
SEGOFF = (0, 8, 16, 24)        # signal offset of each segment


# ---------------------------------------------------------------- host math
def derive_constants(sos: np.ndarray, K: int):
    """Constant matrices for the block SSM, float64. K = chunks per signal."""
    sos = sos.astype(np.float64)
    (b0, b1, b2, a1, a2), (B0, B1, B2, A1, A2) = [
        (s[0] / s[3], s[1] / s[3], s[2] / s[3], s[4] / s[3], s[5] / s[3])
        for s in sos
    ]
    c1, c2 = b1 - b0 * a1, b2 - b0 * a2
    A = np.array([
        [-a1, -a2, 0.0, 0.0],
        [1.0, 0.0, 0.0, 0.0],
        [c1, c2, -A1, -A2],
        [0.0, 0.0, 1.0, 0.0],
    ])
    Bv = np.array([1.0, 0.0, b0, 0.0])
    Cv = np.array([B0 * c1, B0 * c2, B1 - B0 * A1, B2 - B0 * A2])
    D = B0 * b0

    h = np.zeros(L)
    h[0] = D
    s = Bv.copy()
    for t in range(1, L):
        h[t] = Cv @ s
        s = A @ s
    H = np.zeros((L, L))
    for j in range(L):
        H[j:, j] = h[: L - j]

    Fm = np.zeros((4, L))
    Ap = np.eye(4)
    for j in range(L - 1, -1, -1):
        Fm[:, j] = Ap @ Bv
        Ap = A @ Ap
    G = np.zeros((L, 4))
    Ap = np.eye(4)
    for t in range(L):
        G[t, :] = Cv @ Ap
        Ap = A @ Ap

    M = np.linalg.matrix_power(A, L)
    lam, V = np.linalg.eig(M)
    idx = [i for i in range(4) if lam[i].imag > 0]
    assert len(idx) == 2, lam
    lam2, V2 = lam[idx], V[:, idx]
    Vinv2 = np.linalg.inv(V)[idx, :]

    Fmod = Vinv2 @ Fm                      # (2, L) complex
    Fp = np.stack([Fmod[0].real, Fmod[0].imag, Fmod[1].real, Fmod[1].imag])
    GV = G @ V2                            # (L, 2) complex
    Gpp = np.stack([2 * GV[:, 0].real, -2 * GV[:, 0].imag,
                    2 * GV[:, 1].real, -2 * GV[:, 1].imag], axis=1)

    r, th = np.abs(lam2), np.angle(lam2)
    k = np.arange(K)
    # per segment: (mode, signal) block layout, rows 4*off + a*nsig + n
    CCh = np.zeros((4 * NSIG, K), dtype=np.float64)
    SSh = np.zeros((4 * NSIG, K), dtype=np.float64)
    RRh = np.zeros((4 * NSIG, 1), dtype=np.float64)
    for off, ns in zip(SEGOFF, SEG):
        r0 = 4 * off
        for a in range(4):
            e = a // 2
            CCh[r0 + a * ns:r0 + (a + 1) * ns, :] = np.cos(th[e] * k)[None, :]
            SSh[r0 + a * ns:r0 + (a + 1) * ns, :] = \
                (1.0 if a % 2 == 0 else -1.0) * np.sin(th[e] * k)[None, :]
            RRh[r0 + a * ns:r0 + (a + 1) * ns, 0] = r[e]

    f32 = np.float32
    return dict(
        hT=np.ascontiguousarray(H.T, dtype=f32),       # (L, L)  = H^T, rhs
        fT=np.ascontiguousarray(Fp.T, dtype=f32),      # (L, 4)  lhsT F-pass
        gT4=np.ascontiguousarray(Gpp.T, dtype=f32),    # (4, L)  = G''^T, rhs
        cc=np.ascontiguousarray(CCh, dtype=f32),       # (4*NSIG, K)
        ss=np.ascontiguousarray(SSh, dtype=f32),       # (4*HS, K)
        rr=np.ascontiguousarray(np.broadcast_to(RRh, (4 * NSIG, K)),
                                dtype=f32),
        r1=float(r[0]), r2=float(r[1]),
    )


# ---------------------------------------------------------------- program
def build_program(r1: float, r2: float, T: int, nblk: int = 512,
                  loadw: int = 1024, order: str = "A"):
    """Build + compile the per-core Bass program."""
    K = T // L                  # chunks per signal
    COLS = NSIG * K             # total chunk-columns
    assert T % L == 0

    nc = bacc.Bacc("TRN2", target_bir_lowering=False, debug=False,
                   num_devices=N_CORES)
    x_d = nc.dram_tensor("x", [NSIG, T], dt.float32, kind="ExternalInput").ap()
    y_d = nc.dram_tensor("y", [NSIG, T], dt.float32, kind="ExternalOutput").ap()
    ident_d = nc.dram_tensor("ident", [128, 128], dt.float32, kind="ExternalInput").ap()
    hT_d = nc.dram_tensor("hT", [L, L], dt.float32, kind="ExternalInput").ap()
    fT_d = nc.dram_tensor("fT", [L, 4], dt.float32, kind="ExternalInput").ap()
    gT4_d = nc.dram_tensor("gT4", [4, L], dt.float32, kind="ExternalInput").ap()
    cc_d = nc.dram_tensor("cc", [4 * NSIG, K], dt.float32, kind="ExternalInput").ap()
    ss_d = nc.dram_tensor("ss", [4 * NSIG, K], dt.float32, kind="ExternalInput").ap()
    rr_d = nc.dram_tensor("rr", [4 * NSIG, K], dt.float32, kind="ExternalInput").ap()

    x_flat = x_d.rearrange("a b -> (a b)")
    y_flat = y_d.rearrange("a b -> (a b)")

    with tile.TileContext(nc) as tc, ExitStack() as ctx:
        consts = ctx.enter_context(tc.tile_pool(name="consts", bufs=1))
        scanp = ctx.enter_context(tc.tile_pool(name="scan", bufs=1))
        xtp = ctx.enter_context(tc.tile_pool(name="xt", bufs=1))
        ldp = ctx.enter_context(tc.tile_pool(name="ld", bufs=3))
        fsbp = ctx.enter_context(tc.tile_pool(name="fsb", bufs=4))
        sblkp = ctx.enter_context(tc.tile_pool(name="sblk", bufs=1))
        dramp = ctx.enter_context(tc.tile_pool(name="dram", bufs=1, space="DRAM"))
        youtp = ctx.enter_context(tc.tile_pool(name="yout", bufs=3))
        ps_t = ctx.enter_context(tc.tile_pool(name="ps_t", bufs=4, space="PSUM"))
        ps_f = ctx.enter_context(tc.tile_pool(name="ps_f", bufs=2, space="PSUM"))
        ps_y = ctx.enter_context(tc.tile_pool(name="ps_y", bufs=2, space="PSUM"))

        # ---- constants: f32 originals + bf16/f32r device copies
        ident = consts.tile([128, 128], dt.float32)
        nc.scalar.dma_start(ident[:], ident_d[:])
        hT = consts.tile([L, L], dt.float32)
        nc.scalar.dma_start(hT[:], hT_d[:])
        hTb = consts.tile([L, L], dt.float16)
        nc.scalar.copy(hTb[:], hT[:])
        fT = consts.tile([L, 4], dt.float32)
        nc.scalar.dma_start(fT[:], fT_d[:])
        fTb = consts.tile([L, 4], dt.float16)
        nc.scalar.copy(fTb[:], fT[:])
        gT4 = consts.tile([4, L], dt.float32)
        nc.scalar.dma_start(gT4[:], gT4_d[:])
        gT4b = consts.tile([4, L], dt.float16)
        nc.scalar.copy(gT4b[:], gT4[:])
        cc_sz, ss_sz, rr_sz = {}, {}, {}
        for off2, ns2 in zip(SEGOFF, SEG):
            if ns2 in cc_sz:
                continue            # same-size segments share identical blocks
            r0, r1_ = 4 * off2, 4 * off2 + 4 * ns2
            cch = consts.tile([4 * ns2, K], dt.float32, name=f"cc{ns2}")
            nc.scalar.dma_start(cch[:], cc_d[r0:r1_, :])
            ssh = consts.tile([4 * ns2, K], dt.float32, name=f"ss{ns2}")
            nc.scalar.dma_start(ssh[:], ss_d[r0:r1_, :])
            rrh = consts.tile([4 * ns2, K], dt.float32, name=f"rr{ns2}")
            nc.scalar.dma_start(rrh[:], rr_d[r0:r1_, :])
            cc_sz[ns2], ss_sz[ns2], rr_sz[ns2] = cch, ssh, rrh

        Xtb = xtp.tile([128, COLS], dt.float16)    # X^T, time-major, bf16

        tS_tiles = {}
        gb32_tiles = {}
        sblk_tiles = {}
        g_tiles = {}
        eng_ns = {"act": 0.0, "dve": 0.0}

        def bal_copy(out_ap, in_ap, width):
            act_cost = width * 0.833 + 143.0
            dve_cost = width * 1.042 + 125.0
            if eng_ns["act"] + act_cost <= eng_ns["dve"] + dve_cost:
                eng_ns["act"] += act_cost
                nc.scalar.copy(out_ap, in_ap)
            else:
                eng_ns["dve"] += dve_cost
                nc.vector.tensor_copy(out_ap, in_ap)

        def act_copy(out_ap, in_ap, width):
            eng_ns["act"] += width * 0.833 + 143.0
            nc.scalar.copy(out_ap, in_ap)

        def seg_geo(h):
            off, ns = SEGOFF[h], SEG[h]
            return off * K, (off + ns) * K, ns, 4 * off

        def in_phase(h):
            hc0, hc1, ns, crow = seg_geo(h)
            gb32 = dramp.tile([4, ns * K], dt.float32, tag=f"gb32_{h}")
            gb32_tiles[h] = gb32
            FW = 4 * nblk

            state = {"psf": None, "pf_base": 0, "pf_off": 0,
                     "fsb": None, "fsb_base": 0, "fsb_off": 0}

            def f_group(c0, gw):
                """F matmul for cols [c0, c0+gw) + staged copy/bounce."""
                if state["psf"] is not None and state["pf_off"] + gw > nblk:
                    flush_psf()
                if state["psf"] is None:
                    state["psf"] = ps_f.tile([4, nblk], dt.float32,
                                             tag="psf", name="psf")
                    state["pf_base"], state["pf_off"] = c0, 0
                o = state["pf_off"]
                nc.tensor.matmul(state["psf"][:, o:o + gw],
                                 fTb[:], Xtb[:, c0:c0 + gw],
                                 start=True, stop=True)
                state["pf_off"] = o + gw

            def flush_psf():
                if state["psf"] is None:
                    return
                pw = state["pf_off"]
                pb = state["pf_base"]
                if state["fsb"] is not None and state["fsb_off"] + pw > FW:
                    flush_fsb()
                if state["fsb"] is None:
                    state["fsb"] = fsbp.tile([4, FW], dt.float32,
                                             tag="fsb", name="fsb")
                    state["fsb_base"], state["fsb_off"] = pb, 0
                fo = state["fsb_off"]
                bal_copy(state["fsb"][:, fo:fo + pw], state["psf"][:, 0:pw],
                         pw)
                state["fsb_off"] = fo + pw
                state["psf"] = None

            def flush_fsb():
                if state["fsb"] is None:
                    return
                fb, fw = state["fsb_base"], state["fsb_off"]
                nc.gpsimd.dma_start(gb32[:, fb - hc0:fb - hc0 + fw],
                                    state["fsb"][:, 0:fw])
                state["fsb"] = None

            # loads (SP queue), chunk-major tiles
            ld_tiles = []
            r0 = hc0
            while r0 < hc1:
                w = min(loadw, hc1 - r0)
                fullw = (w // 128) * 128
                if fullw:
                    t_in = ldp.tile([128, loadw], dt.float32, tag="ld")
                    view = x_flat[r0 * 128:(r0 + fullw) * 128].rearrange(
                        "(q p t) -> p q t", p=128, t=128)
                    dst = t_in[:, 0:fullw].rearrange("p (q t) -> p q t", t=128)
                    nc.sync.dma_start(dst, view)
                    ld_tiles.append((t_in, r0, fullw))
                if w > fullw:
                    rem = w - fullw
                    t_in2 = ldp.tile([128, 128], dt.float32, tag="ldp")
                    view = x_flat[(r0 + fullw) * 128:(r0 + w) * 128].rearrange(
                        "(p t) -> p t", p=rem, t=128)
                    nc.sync.dma_start(t_in2[0:rem, :], view)
                    ld_tiles.append((t_in2, r0 + fullw, rem))
                r0 += w

            # PE transposes -> psum -> copy -> Xtb (fp16), F per group
            for t_in, col0, width in ld_tiles:
                if width >= 128:
                    for g0 in range(0, width, 512):
                        gw = min(512, width - g0)
                        pst = ps_t.tile([128, 512], dt.float32, tag="pst")
                        for q in range(gw // 128):
                            nc.tensor.transpose(
                                pst[:, q * 128:(q + 1) * 128],
                                t_in[:, g0 + q * 128:g0 + (q + 1) * 128],
                                ident[:])
                        bal_copy(Xtb[:, col0 + g0:col0 + g0 + gw],
                                 pst[:, 0:gw], gw)
                        f_group(col0 + g0, gw)
                else:
                    pst2 = ps_t.tile([128, 512], dt.float32, tag="pst")
                    nc.tensor.transpose(pst2[0:128, 0:width],
                                        t_in[0:width, 0:128],
                                        ident[0:width, 0:width])
                    bal_copy(Xtb[:, col0:col0 + width],
                             pst2[:, 0:width], width)
                    f_group(col0, width)
            flush_psf()
            flush_fsb()

        def scan_dma(h):
            hc0, hc1, ns, crow = seg_geo(h)
            gb32 = gb32_tiles[h]
            gbv = gb32[:].rearrange("a c -> (a c)").rearrange(
                "(r k) -> r k", k=K)
            # g_t[a*ns+n, k] = gb32[a, n*K+k]   (ACT queue)
            gtag = "A" if h % 2 == 0 else "B"
            g_t = scanp.tile([4 * max(SEG), K], dt.float32,
                             tag=f"g_t{gtag}", name="g_t")
            nc.gpsimd.dma_start(g_t[0:4 * ns, :], gbv)
            # gswap = mode-pair swap of g_t
            gswap = scanp.tile([4 * max(SEG), K], dt.float32,
                               tag=f"gswap{gtag}", name="gswap")
            for (d0, s0) in ((0, ns), (ns, 0), (2 * ns, 3 * ns),
                             (3 * ns, 2 * ns)):
                nc.gpsimd.dma_start(gswap[d0:d0 + ns, :],
                                    gbv[s0:s0 + ns, :])
            g_tiles[h] = (g_t, gswap)

        def scan_ops(h):
            hc0, hc1, ns, crow = seg_geo(h)
            g_t, gswap = g_tiles[h]
            ns_ = SEG[h]
            ccs = cc_sz[ns_][:]
            sss = ss_sz[ns_][:]
            rrs = rr_sz[ns_][:]
            # rotation trick:  gt_tw = cc*g + ss*gswap ;  m = scan(r, gt_tw)
            # tS[:, k+1] = cc*m - ss*swap(m)
            gt_tw = scanp.tile([4 * ns, K], dt.float32, tag="gt_tw",
                               name="gt_tw")
            tmp1 = scanp.tile([4 * ns, K], dt.float32, tag="scr1",
                              name="tmp1")
            tmp2 = scanp.tile([4 * ns, K], dt.float32, tag="scr2",
                              name="tmp2")
            nc.vector.tensor_mul(tmp1[:], ccs, g_t[0:4 * ns, :])
            nc.vector.tensor_mul(tmp2[:], sss, gswap[0:4 * ns, :])
            nc.vector.tensor_add(gt_tw[:], tmp1[:], tmp2[:])
            m_t = scanp.tile([4 * ns, K], dt.float32, tag="m_t", name="m_t")
            nc.vector.tensor_tensor_scan(
                m_t[:], rrs, gt_tw[:], 0.0,
                mybir.AluOpType.mult, mybir.AluOpType.add)
            mswap = scanp.tile([4 * ns, K], dt.float32, tag="mswap",
                               name="mswap")
            for (d0, s0) in ((0, ns), (ns, 0), (2 * ns, 3 * ns),
                             (3 * ns, 2 * ns)):
                nc.gpsimd.dma_start(mswap[d0:d0 + ns, :],
                                    m_t[s0:s0 + ns, :])
            tS = scanp.tile([4 * ns, K + 1], dt.float16, tag=f"tS{h}",
                            name="tS")
            tS_tiles[h] = tS
            nc.vector.memset(tS[:, 0:1], 0.0)
            t1b = scanp.tile([4 * ns, K], dt.float32, tag="scr1", name="t1b")
            t2b = scanp.tile([4 * ns, K], dt.float32, tag="scr2", name="t2b")
            nc.vector.tensor_mul(t1b[:], ccs, m_t[:])
            nc.vector.tensor_mul(t2b[:], sss, mswap[:])
            nc.vector.tensor_sub(tS[:, 1:K + 1], t1b[:], t2b[:])
            eng_ns["dve"] += 7 * (K * 1.042 + 125.0)
            gtag2 = "A" if h % 2 == 0 else "B"
            # stage S into G-pass lhsT layout: sblk[a, n*K+k] = tS[a*ns+n, k]
            sblk = sblkp.tile([4, max(SEG) * K], dt.float16,
                              tag=f"sblk{gtag2}", name="sblk")
            sblk_tiles[h] = sblk
            for n in range(ns):
                nc.sync.dma_start(sblk[:, n * K:(n + 1) * K],
                                  tS[n::ns, 0:K])

        def out_phase(h):
            hc0, hc1, ns, crow = seg_geo(h)
            sblk = sblk_tiles[h]
            storew = 1024
            yout, yo_base = None, 0
            c0 = hc0
            while c0 < hc1:
                c1 = min(c0 + nblk, hc1)
                n = c1 - c0
                if yout is None:
                    yout = youtp.tile([128, storew], dt.float32, tag="yout")
                    yo_base = c0
                psy = ps_y.tile([128, 512], dt.float32, tag="psy")
                # per 128-col slab: Y[col, t] = Xslab^T H^T + Sslab^T G''^T
                q = 0
                while q * 128 < n:
                    w = min(128, n - q * 128)
                    sc0 = c0 + q * 128
                    nc.tensor.matmul(psy[0:w, q * 128:q * 128 + 128],
                                     Xtb[:, sc0:sc0 + w], hTb[:],
                                     start=True, stop=False)
                    nc.tensor.matmul(psy[0:w, q * 128:q * 128 + 128],
                                     sblk[:, sc0 - hc0:sc0 - hc0 + w],
                                     gT4b[:],
                                     start=False, stop=True)
                    q += 1
                fullq = n // 128
                off0 = c0 - yo_base
                if fullq:
                    bal_copy(yout[:, off0:off0 + fullq * 128],
                             psy[:, 0:fullq * 128], fullq * 128)
                if n > fullq * 128:
                    w = n - fullq * 128
                    bal_copy(
                        yout[0:w, off0 + fullq * 128:off0 + fullq * 128 + 128],
                        psy[0:w, fullq * 128:fullq * 128 + 128], 128)

                if c1 - yo_base >= storew or c1 == hc1:
                    wq = c1 - yo_base
                    fullw = (wq // 128) * 128
                    if fullw:
                        view = y_flat[yo_base * 128:(yo_base + fullw) * 128] \
                            .rearrange("(qq p t) -> p qq t", p=128, t=128)
                        srcv = yout[:, 0:fullw].rearrange(
                            "p (qq t) -> p qq t", t=128)
                        nc.scalar.dma_start(view, srcv)
                    if wq > fullw:
                        rem = wq - fullw
                        view = y_flat[(yo_base + fullw) * 128:
                                      (yo_base + wq) * 128].rearrange(
                            "(p t) -> p t", p=rem, t=128)
                        nc.scalar.dma_start(view, yout[0:rem,
                                                       fullw:fullw + 128])
                    yout = None
                c0 = c1

        # software pipeline: per-engine FIFO order must match dependency
        # readiness (in-order SEQs); g-loads precede older stores on ACT
        in_phase(0)
        scan_dma(0)
        in_phase(1)
        scan_ops(0)
        scan_dma(1)
        in_phase(2)
        scan_ops(1)
        out_phase(0)
        scan_dma(2)
        in_phase(3)
        scan_ops(2)
        out_phase(1)
        scan_dma(3)
        scan_ops(3)
        out_phase(2)
        out_phase(3)
    nc.compile()
    return nc


class _Exec:
    """Cached PJRT executable for one built program (8-core shard_map)."""

    def __init__(self, nc):
        import jax
        import jax.numpy as jnp
        from jax.sharding import Mesh, PartitionSpec, NamedSharding
        try:
            from jax.experimental.shard_map import shard_map
        except ImportError:
            from jax import shard_map
        from concourse import bass2jax
        from concourse.bass2jax import _bass_exec_p, partition_id_tensor

        bass2jax.install_neuronx_cc_hook()
        assert nc.dbg_addr is None
        pname = nc.partition_id_tensor.name if nc.partition_id_tensor else None
        in_names, out_names, out_avals, zero_outs = [], [], [], []
        for alloc in nc.m.functions[0].allocations:
            if not isinstance(alloc, mybir.MemoryLocationSet):
                continue
            name = alloc.memorylocations[0].name
            if alloc.kind == "ExternalInput":
                if name != pname:
                    in_names.append(name)
            elif alloc.kind == "ExternalOutput":
                shape = tuple(alloc.tensor_shape)
                dtype = mybir.dt.np(alloc.dtype)
                out_names.append(name)
                out_avals.append(jax.core.ShapedArray(shape, dtype))
                zero_outs.append(np.zeros(shape, dtype))
        n_params = len(in_names)
        all_in = in_names + out_names + ([pname] if pname else [])

        def _body(*args):
            operands = list(args)
            if pname is not None:
                operands.append(partition_id_tensor())
            return tuple(_bass_exec_p.bind(
                *operands,
                out_avals=tuple(out_avals),
                in_names=tuple(all_in),
                out_names=tuple(out_names),
                lowering_input_output_aliases=(),
                sim_require_finite=True,
                sim_require_nnan=True,
                nc=nc,
            ))

        devices = jax.devices()[:N_CORES]
        self.mesh = Mesh(np.asarray(devices), ("core",))
        nin = n_params + len(zero_outs)
        self.fn = jax.jit(shard_map(
            _body, mesh=self.mesh,
            in_specs=(PartitionSpec("core"),) * nin,
            out_specs=(PartitionSpec("core"),) * len(out_names),
            check_rep=False))
        self.sharding = NamedSharding(self.mesh, PartitionSpec("core"))
        self.in_names, self.out_names = in_names, out_names
        self.out_avals, self.zero_outs = out_avals, zero_outs
        self.jax, self.jnp = jax, jnp

    def stage(self, in_maps):
        """device_put concat inputs + zero outs; returns arg list."""
        jax = self.jax
        args = []
        for i, name in enumerate(self.in_names):
            cat = np.concatenate([np.asarray(m[name]) for m in in_maps], 0)
            args.append(jax.device_put(cat, self.sharding))
        for z in self.zero_outs:
            zz = np.zeros((N_CORES * z.shape[0], *z.shape[1:]), z.dtype)
            args.append(jax.device_put(zz, self.sharding))
        return args

    def __call__(self, args):
        outs = self.fn(*args)
        self.jax.block_until_ready(outs)
        return outs


_CACHE: dict = {}


def _get_exec(key, r1, r2, T, nblk, loadw):
    if key not in _CACHE:
        nc = build_program(r1, r2, T, nblk=nblk, loadw=loadw)
        _CACHE[key] = (nc, _Exec(nc))
    return _CACHE[key]


def run_filter(x: np.ndarray, sos: np.ndarray, T: int = T_FULL,
               nblk: int = 512, loadw: int = 1024, time_reps: int = 0):
    """x: (256, T) float32 -> (y (256, T) float32, times list[s])."""
    import time as _time
    K = T // L
    consts = derive_constants(sos, K)
    key = (sos.astype(np.float32).tobytes(), T, nblk, loadw)
    nc, ex = _get_exec(key, consts["r1"], consts["r2"], T, nblk, loadw)

    shards = x.reshape(N_CORES, NSIG, T)
    base = {k: consts[k] for k in ("hT", "fT", "gT4", "cc", "ss", "rr")}
    base["ident"] = np.eye(128, dtype=np.float32)
    in_maps = [dict(base, x=np.ascontiguousarray(shards[i]))
               for i in range(N_CORES)]
    args = ex.stage(in_maps)
    outs = ex(args)                       # first call compiles + runs
    times = []
    for _ in range(time_reps):
        t0 = _time.perf_counter()
        outs2 = ex(args)
        times.append(_time.perf_counter() - t0)
    oi = ex.out_names.index("y")
    y = np.asarray(outs[oi]).reshape(N_CORES * NSIG, T)
    return y, times


def kernel(x: np.ndarray, sos: np.ndarray) -> np.ndarray:
    x = np.asarray(x, dtype=np.float32)
    sos = np.asarray(sos, dtype=np.float32)
    y, _ = run_filter(x.reshape(B * C, T_FULL), sos)
    return y.reshape(B, C, T_FULL).astype(np.float32)


# revision 53
# speedup vs baseline: 1.0651x; 1.0651x over previous
"""Butterworth 4th-order lowpass (2 cascaded biquads) on 8 TRN2 NeuronCores.

Algorithm: block state-space decomposition of the IIR cascade.
  - Chunk the time axis into L=128 blocks. Within a chunk, the zero-state
    response is a lower-triangular Toeplitz matmul; chunk-boundary states
    follow s_k = M s_{k-1} + f_k with M = A^L, diagonalized into two
    complex modes solved by first-order REAL scans (DVE tensor_tensor_scan)
    via the rotation trick.
  - The output is computed directly in chunk-row-major layout as
       Y[col, t] = (X^T)^T H^T + S^T G''^T
    i.e. matmul with lhsT = time-major X slab (bf16) and rhs = H^T (bf16),
    plus an accumulated state-correction matmul (lhsT = S, rhs = G''^T).
    bf16 matmuls run 1 cycle/row even at 128-wide outputs, and the result
    lands store-ready (no output transpose pass).
  - The state path (F-pass projection, scan) stays f32 for accuracy.
Sharding: 256 independent signals, 32 per core, no cross-core comm.
"""
import numpy as np
from contextlib import ExitStack

import concourse.bass as bass
import concourse.tile as tile
from concourse import bacc, mybir
from concourse.bass_utils import run_bass_kernel_spmd

dt = mybir.dt

B, C, T_FULL = 32, 8, 96000
N_CORES = 8
NSIG = (B * C) // N_CORES      # 32 signals per core
L = 128                        # chunk length
SEG = (8, 8, 8, 8)             # segment sizes; 4*size and 4*offset must be 32-aligned
SEGOFF = (0, 8, 16, 24)        # signal offset of each segment


# ---------------------------------------------------------------- host math
def derive_constants(sos: np.ndarray, K: int):
    """Constant matrices for the block SSM, float64. K = chunks per signal."""
    sos = sos.astype(np.float64)
    (b0, b1, b2, a1, a2), (B0, B1, B2, A1, A2) = [
        (s[0] / s[3], s[1] / s[3], s[2] / s[3], s[4] / s[3], s[5] / s[3])
        for s in sos
    ]
    c1, c2 = b1 - b0 * a1, b2 - b0 * a2
    A = np.array([
        [-a1, -a2, 0.0, 0.0],
        [1.0, 0.0, 0.0, 0.0],
        [c1, c2, -A1, -A2],
        [0.0, 0.0, 1.0, 0.0],
    ])
    Bv = np.array([1.0, 0.0, b0, 0.0])
    Cv = np.array([B0 * c1, B0 * c2, B1 - B0 * A1, B2 - B0 * A2])
    D = B0 * b0

    h = np.zeros(L)
    h[0] = D
    s = Bv.copy()
    for t in range(1, L):
        h[t] = Cv @ s
        s = A @ s
    H = np.zeros((L, L))
    for j in range(L):
        H[j:, j] = h[: L - j]

    Fm = np.zeros((4, L))
    Ap = np.eye(4)
    for j in range(L - 1, -1, -1):
        Fm[:, j] = Ap @ Bv
        Ap = A @ Ap
    G = np.zeros((L, 4))
    Ap = np.eye(4)
    for t in range(L):
        G[t, :] = Cv @ Ap
        Ap = A @ Ap

    M = np.linalg.matrix_power(A, L)
    lam, V = np.linalg.eig(M)
    idx = [i for i in range(4) if lam[i].imag > 0]
    assert len(idx) == 2, lam
    lam2, V2 = lam[idx], V[:, idx]
    Vinv2 = np.linalg.inv(V)[idx, :]

    Fmod = Vinv2 @ Fm                      # (2, L) complex
    Fp = np.stack([Fmod[0].real, Fmod[0].imag, Fmod[1].real, Fmod[1].imag])
    GV = G @ V2                            # (L, 2) complex
    Gpp = np.stack([2 * GV[:, 0].real, -2 * GV[:, 0].imag,
                    2 * GV[:, 1].real, -2 * GV[:, 1].imag], axis=1)

    r, th = np.abs(lam2), np.angle(lam2)
    k = np.arange(K)
    # per segment: (mode, signal) block layout, rows 4*off + a*nsig + n
    CCh = np.zeros((4 * NSIG, K), dtype=np.float64)
    SSh = np.zeros((4 * NSIG, K), dtype=np.float64)
    RRh = np.zeros((4 * NSIG, 1), dtype=np.float64)
    for off, ns in zip(SEGOFF, SEG):
        r0 = 4 * off
        for a in range(4):
            e = a // 2
            CCh[r0 + a * ns:r0 + (a + 1) * ns, :] = np.cos(th[e] * k)[None, :]
            SSh[r0 + a * ns:r0 + (a + 1) * ns, :] = \
                (1.0 if a % 2 == 0 else -1.0) * np.sin(th[e] * k)[None, :]
            RRh[r0 + a * ns:r0 + (a + 1) * ns, 0] = r[e]

    f32 = np.float32
    return dict(
        hT=np.ascontiguousarray(H.T, dtype=f32),       # (L, L)  = H^T, rhs
        fT=np.ascontiguousarray(Fp.T, dtype=f32),      # (L, 4)  lhsT F-pass
        gT4=np.ascontiguousarray(Gpp.T, dtype=f32),    # (4, L)  = G''^T, rhs
        cc=np.ascontiguousarray(CCh, dtype=f32),       # (4*NSIG, K)
        ss=np.ascontiguousarray(SSh, dtype=f32),       # (4*HS, K)
        rr=np.ascontiguousarray(np.broadcast_to(RRh, (4 * NSIG, K)),
                                dtype=f32),
        r1=float(r[0]), r2=float(r[1]),
    )


# ---------------------------------------------------------------- program
def build_program(r1: float, r2: float, T: int, nblk: int = 512,
                  loadw: int = 1024, order: str = "A"):
    """Build + compile the per-core Bass program."""
    K = T // L                  # chunks per signal
    COLS = NSIG * K             # total chunk-columns
    assert T % L == 0

    nc = bacc.Bacc("TRN2", target_bir_lowering=False, debug=False,
                   num_devices=N_CORES)
    x_d = nc.dram_tensor("x", [NSIG, T], dt.float32, kind="ExternalInput").ap()
    y_d = nc.dram_tensor("y", [NSIG, T], dt.float32, kind="ExternalOutput").ap()
    ident_d = nc.dram_tensor("ident", [128, 128], dt.float32, kind="ExternalInput").ap()
    hT_d = nc.dram_tensor("hT", [L, L], dt.float32, kind="ExternalInput").ap()
    fT_d = nc.dram_tensor("fT", [L, 4], dt.float32, kind="ExternalInput").ap()
    gT4_d = nc.dram_tensor("gT4", [4, L], dt.float32, kind="ExternalInput").ap()
    cc_d = nc.dram_tensor("cc", [4 * NSIG, K], dt.float32, kind="ExternalInput").ap()
    ss_d = nc.dram_tensor("ss", [4 * NSIG, K], dt.float32, kind="ExternalInput").ap()
    rr_d = nc.dram_tensor("rr", [4 * NSIG, K], dt.float32, kind="ExternalInput").ap()

    x_flat = x_d.rearrange("a b -> (a b)")
    y_flat = y_d.rearrange("a b -> (a b)")

    with tile.TileContext(nc) as tc, ExitStack() as ctx:
        consts = ctx.enter_context(tc.tile_pool(name="consts", bufs=1))
        scanp = ctx.enter_context(tc.tile_pool(name="scan", bufs=1))
        xtp = ctx.enter_context(tc.tile_pool(name="xt", bufs=1))
        ldp = ctx.enter_context(tc.tile_pool(name="ld", bufs=3))
        fsbp = ctx.enter_context(tc.tile_pool(name="fsb", bufs=4))
        sblkp = ctx.enter_context(tc.tile_pool(name="sblk", bufs=1))
        dramp = ctx.enter_context(tc.tile_pool(name="dram", bufs=1, space="DRAM"))
        youtp = ctx.enter_context(tc.tile_pool(name="yout", bufs=3))
        ps_t = ctx.enter_context(tc.tile_pool(name="ps_t", bufs=4, space="PSUM"))
        ps_f = ctx.enter_context(tc.tile_pool(name="ps_f", bufs=2, space="PSUM"))
        ps_y = ctx.enter_context(tc.tile_pool(name="ps_y", bufs=2, space="PSUM"))

        # ---- constants: f32 originals + bf16/f32r device copies
        ident = consts.tile([128, 128], dt.float32)
        nc.scalar.dma_start(ident[:], ident_d[:])
        hT = consts.tile([L, L], dt.float32)
        nc.scalar.dma_start(hT[:], hT_d[:])
        hTb = consts.tile([L, L], dt.float16)
        nc.scalar.copy(hTb[:], hT[:])
        fT = consts.tile([L, 4], dt.float32)
        nc.scalar.dma_start(fT[:], fT_d[:])
        fTb = consts.tile([L, 4], dt.float16)
        nc.scalar.copy(fTb[:], fT[:])
        gT4 = consts.tile([4, L], dt.float32)
        nc.scalar.dma_start(gT4[:], gT4_d[:])
        gT4b = consts.tile([4, L], dt.float16)
        nc.scalar.copy(gT4b[:], gT4[:])
        cc_sz, ss_sz, rr_sz = {}, {}, {}
        for off2, ns2 in zip(SEGOFF, SEG):
            if ns2 in cc_sz:
                continue            # same-size segments share identical blocks
            r0, r1_ = 4 * off2, 4 * off2 + 4 * ns2
            cch = consts.tile([4 * ns2, K], dt.float32, name=f"cc{ns2}")
            nc.scalar.dma_start(cch[:], cc_d[r0:r1_, :])
            ssh = consts.tile([4 * ns2, K], dt.float32, name=f"ss{ns2}")
            nc.scalar.dma_start(ssh[:], ss_d[r0:r1_, :])
            rrh = consts.tile([4 * ns2, K], dt.float32, name=f"rr{ns2}")
            nc.scalar.dma_start(rrh[:], rr_d[r0:r1_, :])
            cc_sz[ns2], ss_sz[ns2], rr_sz[ns2] = cch, ssh, rrh

        Xtb = xtp.tile([128, COLS], dt.float16)    # X^T, time-major, bf16

        tS_tiles = {}
        gb32_tiles = {}
        sblk_tiles = {}
        g_tiles = {}
        eng_ns = {"act": 0.0, "dve": 0.0}

        def bal_copy(out_ap, in_ap, width):
            act_cost = width * 0.833 + 143.0
            dve_cost = width * 1.042 + 125.0
            if eng_ns["act"] + act_cost <= eng_ns["dve"] + dve_cost:
                eng_ns["act"] += act_cost
                nc.scalar.copy(out_ap, in_ap)
            else:
                eng_ns["dve"] += dve_cost
                nc.vector.tensor_copy(out_ap, in_ap)

        def act_copy(out_ap, in_ap, width):
            eng_ns["act"] += width * 0.833 + 143.0
            nc.scalar.copy(out_ap, in_ap)

        def seg_geo(h):
            off, ns = SEGOFF[h], SEG[h]
            return off * K, (off + ns) * K, ns, 4 * off

        def in_phase(h):
            hc0, hc1, ns, crow = seg_geo(h)
            gb32 = dramp.tile([4, ns * K], dt.float32, tag=f"gb32_{h}")
            gb32_tiles[h] = gb32
            FW = 4 * nblk

            state = {"psf": None, "pf_base": 0, "pf_off": 0,
                     "fsb": None, "fsb_base": 0, "fsb_off": 0}

            def f_group(c0, gw):
                """F matmul for cols [c0, c0+gw) + staged copy/bounce."""
                if state["psf"] is not None and state["pf_off"] + gw > nblk:
                    flush_psf()
                if state["psf"] is None:
                    state["psf"] = ps_f.tile([4, nblk], dt.float32,
                                             tag="psf", name="psf")
                    state["pf_base"], state["pf_off"] = c0, 0
                o = state["pf_off"]
                nc.tensor.matmul(state["psf"][:, o:o + gw],
                                 fTb[:], Xtb[:, c0:c0 + gw],
                                 start=True, stop=True)
                state["pf_off"] = o + gw

            def flush_psf():
                if state["psf"] is None:
                    return
                pw = state["pf_off"]
                pb = state["pf_base"]
                if state["fsb"] is not None and state["fsb_off"] + pw > FW:
                    flush_fsb()
                if state["fsb"] is None:
                    state["fsb"] = fsbp.tile([4, FW], dt.float32,
                                             tag="fsb", name="fsb")
                    state["fsb_base"], state["fsb_off"] = pb, 0
                fo = state["fsb_off"]
                bal_copy(state["fsb"][:, fo:fo + pw], state["psf"][:, 0:pw],
                         pw)
                state["fsb_off"] = fo + pw
                state["psf"] = None

            def flush_fsb():
                if state["fsb"] is None:
                    return
                fb, fw = state["fsb_base"], state["fsb_off"]
                nc.gpsimd.dma_start(gb32[:, fb - hc0:fb - hc0 + fw],
                                    state["fsb"][:, 0:fw])
                state["fsb"] = None

            # loads (SP queue), chunk-major tiles
            ld_tiles = []
            r0 = hc0
            while r0 < hc1:
                w = min(loadw, hc1 - r0)
                fullw = (w // 128) * 128
                if fullw:
                    t_in = ldp.tile([128, loadw], dt.float32, tag="ld")
                    view = x_flat[r0 * 128:(r0 + fullw) * 128].rearrange(
                        "(q p t) -> p q t", p=128, t=128)
                    dst = t_in[:, 0:fullw].rearrange("p (q t) -> p q t", t=128)
                    nc.sync.dma_start(dst, view)
                    ld_tiles.append((t_in, r0, fullw))
                if w > fullw:
                    rem = w - fullw
                    t_in2 = ldp.tile([128, 128], dt.float32, tag="ldp")
                    view = x_flat[(r0 + fullw) * 128:(r0 + w) * 128].rearrange(
                        "(p t) -> p t", p=rem, t=128)
                    nc.sync.dma_start(t_in2[0:rem, :], view)
                    ld_tiles.append((t_in2, r0 + fullw, rem))
                r0 += w

            # PE transposes -> psum -> copy -> Xtb (fp16), F per group
            for t_in, col0, width in ld_tiles:
                if width >= 128:
                    for g0 in range(0, width, 512):
                        gw = min(512, width - g0)
                        pst = ps_t.tile([128, 512], dt.float32, tag="pst")
                        for q in range(gw // 128):
                            nc.tensor.transpose(
                                pst[:, q * 128:(q + 1) * 128],
                                t_in[:, g0 + q * 128:g0 + (q + 1) * 128],
                                ident[:])
                        bal_copy(Xtb[:, col0 + g0:col0 + g0 + gw],
                                 pst[:, 0:gw], gw)
                        f_group(col0 + g0, gw)
                else:
                    pst2 = ps_t.tile([128, 512], dt.float32, tag="pst")
                    nc.tensor.transpose(pst2[0:128, 0:width],
                                        t_in[0:width, 0:128],
                                        ident[0:width, 0:width])
                    bal_copy(Xtb[:, col0:col0 + width],
                             pst2[:, 0:width], width)
                    f_group(col0, width)
            flush_psf()
            flush_fsb()

        def scan_dma(h):
            hc0, hc1, ns, crow = seg_geo(h)
            gb32 = gb32_tiles[h]
            gbv = gb32[:].rearrange("a c -> (a c)").rearrange(
                "(r k) -> r k", k=K)
            # g_t[a*ns+n, k] = gb32[a, n*K+k]   (ACT queue)
            gtag = "A" if h % 2 == 0 else "B"
            g_t = scanp.tile([4 * max(SEG), K], dt.float32,
                             tag=f"g_t{gtag}", name="g_t")
            nc.gpsimd.dma_start(g_t[0:4 * ns, :], gbv)
            # gswap = mode-pair swap of g_t
            gswap = scanp.tile([4 * max(SEG), K], dt.float32,
                               tag=f"gswap{gtag}", name="gswap")
            for (d0, s0) in ((0, ns), (ns, 0), (2 * ns, 3 * ns),
                             (3 * ns, 2 * ns)):
                nc.gpsimd.dma_start(gswap[d0:d0 + ns, :],
                                    gbv[s0:s0 + ns, :])
            g_tiles[h] = (g_t, gswap)

        def scan_ops(h):
            hc0, hc1, ns, crow = seg_geo(h)
            g_t, gswap = g_tiles[h]
            ns_ = SEG[h]
            ccs = cc_sz[ns_][:]
            sss = ss_sz[ns_][:]
            rrs = rr_sz[ns_][:]
            # rotation trick:  gt_tw = cc*g + ss*gswap ;  m = scan(r, gt_tw)
            # tS[:, k+1] = cc*m - ss*swap(m)
            gt_tw = scanp.tile([4 * ns, K], dt.float32, tag="gt_tw",
                               name="gt_tw")
            tmp1 = scanp.tile([4 * ns, K], dt.float32, tag="scr1",
                              name="tmp1")
            tmp2 = scanp.tile([4 * ns, K], dt.float32, tag="scr2",
                              name="tmp2")
            nc.vector.tensor_mul(tmp1[:], ccs, g_t[0:4 * ns, :])
            nc.vector.tensor_mul(tmp2[:], sss, gswap[0:4 * ns, :])
            nc.vector.tensor_add(gt_tw[:], tmp1[:], tmp2[:])
            m_t = scanp.tile([4 * ns, K], dt.float32, tag="m_t", name="m_t")
            nc.vector.tensor_tensor_scan(
                m_t[:], rrs, gt_tw[:], 0.0,
                mybir.AluOpType.mult, mybir.AluOpType.add)
            mswap = scanp.tile([4 * ns, K], dt.float32, tag="mswap",
                               name="mswap")
            for (d0, s0) in ((0, ns), (ns, 0), (2 * ns, 3 * ns),
                             (3 * ns, 2 * ns)):
                nc.gpsimd.dma_start(mswap[d0:d0 + ns, :],
                                    m_t[s0:s0 + ns, :])
            tS = scanp.tile([4 * ns, K + 1], dt.float16, tag=f"tS{h}",
                            name="tS")
            tS_tiles[h] = tS
            nc.vector.memset(tS[:, 0:1], 0.0)
            t1b = scanp.tile([4 * ns, K], dt.float32, tag="scr1", name="t1b")
            t2b = scanp.tile([4 * ns, K], dt.float32, tag="scr2", name="t2b")
            nc.vector.tensor_mul(t1b[:], ccs, m_t[:])
            nc.vector.tensor_mul(t2b[:], sss, mswap[:])
            nc.vector.tensor_sub(tS[:, 1:K + 1], t1b[:], t2b[:])
            eng_ns["dve"] += 7 * (K * 1.042 + 125.0)
            gtag2 = "A" if h % 2 == 0 else "B"
            # stage S into G-pass lhsT layout: sblk[a, n*K+k] = tS[a*ns+n, k]
            sblk = sblkp.tile([4, max(SEG) * K], dt.float16,
                              tag=f"sblk{gtag2}", name="sblk")
            sblk_tiles[h] = sblk
            for n in range(ns):
                nc.sync.dma_start(sblk[:, n * K:(n + 1) * K],
                                  tS[n::ns, 0:K])

        def out_phase(h):
            hc0, hc1, ns, crow = seg_geo(h)
            sblk = sblk_tiles[h]
            storew = 1024
            yout, yo_base = None, 0
            c0 = hc0
            while c0 < hc1:
                c1 = min(c0 + nblk, hc1)
                n = c1 - c0
                if yout is None:
                    yout = youtp.tile([128, storew], dt.float32, tag="yout")
                    yo_base = c0
                psy = ps_y.tile([128, 512], dt.float32, tag="psy")
                # per 128-col slab: Y[col, t] = Xslab^T H^T + Sslab^T G''^T
                q = 0
                while q * 128 < n:
                    w = min(128, n - q * 128)
                    sc0 = c0 + q * 128
                    nc.tensor.matmul(psy[0:w, q * 128:q * 128 + 128],
                                     Xtb[:, sc0:sc0 + w], hTb[:],
                                     start=True, stop=False)
                    nc.tensor.matmul(psy[0:w, q * 128:q * 128 + 128],
                                     sblk[:, sc0 - hc0:sc0 - hc0 + w],
                                     gT4b[:],
                                     start=False, stop=True)
                    q += 1
                fullq = n // 128
                off0 = c0 - yo_base
                if fullq:
                    bal_copy(yout[:, off0:off0 + fullq * 128],
                             psy[:, 0:fullq * 128], fullq * 128)
                if n > fullq * 128:
                    w = n - fullq * 128
                    bal_copy(
                        yout[0:w, off0 + fullq * 128:off0 + fullq * 128 + 128],
                        psy[0:w, fullq * 128:fullq * 128 + 128], 128)

                if c1 - yo_base >= storew or c1 == hc1:
                    wq = c1 - yo_base
                    fullw = (wq // 128) * 128
                    if fullw:
                        view = y_flat[yo_base * 128:(yo_base + fullw) * 128] \
                            .rearrange("(qq p t) -> p qq t", p=128, t=128)
                        srcv = yout[:, 0:fullw].rearrange(
                            "p (qq t) -> p qq t", t=128)
                        nc.scalar.dma_start(view, srcv)
                    if wq > fullw:
                        rem = wq - fullw
                        view = y_flat[(yo_base + fullw) * 128:
                                      (yo_base + wq) * 128].rearrange(
                            "(p t) -> p t", p=rem, t=128)
                        nc.scalar.dma_start(view, yout[0:rem,
                                                       fullw:fullw + 128])
                    yout = None
                c0 = c1

        # software pipeline: per-engine FIFO order must match dependency
        # readiness (in-order SEQs); g-loads precede older stores on ACT
        in_phase(0)
        scan_dma(0)
        in_phase(1)
        scan_ops(0)
        scan_dma(1)
        in_phase(2)
        scan_ops(1)
        out_phase(0)
        scan_dma(2)
        in_phase(3)
        scan_ops(2)
        out_phase(1)
        scan_dma(3)
        scan_ops(3)
        out_phase(2)
        out_phase(3)
    nc.compile()
    return nc


class _Exec:
    """Cached PJRT executable for one built program (8-core shard_map)."""

    def __init__(self, nc):
        import jax
        import jax.numpy as jnp
        from jax.sharding import Mesh, PartitionSpec, NamedSharding
        try:
            from jax.experimental.shard_map import shard_map
        except ImportError:
            from jax import shard_map
        from concourse import bass2jax
        from concourse.bass2jax import _bass_exec_p, partition_id_tensor

        bass2jax.install_neuronx_cc_hook()
        assert nc.dbg_addr is None
        pname = nc.partition_id_tensor.name if nc.partition_id_tensor else None
        in_names, out_names, out_avals, zero_outs = [], [], [], []
        for alloc in nc.m.functions[0].allocations:
            if not isinstance(alloc, mybir.MemoryLocationSet):
                continue
            name = alloc.memorylocations[0].name
            if alloc.kind == "ExternalInput":
                if name != pname:
                    in_names.append(name)
            elif alloc.kind == "ExternalOutput":
                shape = tuple(alloc.tensor_shape)
                dtype = mybir.dt.np(alloc.dtype)
                out_names.append(name)
                out_avals.append(jax.core.ShapedArray(shape, dtype))
                zero_outs.append(np.zeros(shape, dtype))
        n_params = len(in_names)
        all_in = in_names + out_names + ([pname] if pname else [])

        def _body(*args):
            operands = list(args)
            if pname is not None:
                operands.append(partition_id_tensor())
            return tuple(_bass_exec_p.bind(
                *operands,
                out_avals=tuple(out_avals),
                in_names=tuple(all_in),
                out_names=tuple(out_names),
                lowering_input_output_aliases=(),
                sim_require_finite=True,
                sim_require_nnan=True,
                nc=nc,
            ))

        devices = jax.devices()[:N_CORES]
        self.mesh = Mesh(np.asarray(devices), ("core",))
        nin = n_params + len(zero_outs)
        self.fn = jax.jit(shard_map(
            _body, mesh=self.mesh,
            in_specs=(PartitionSpec("core"),) * nin,
            out_specs=(PartitionSpec("core"),) * len(out_names),
            check_rep=False))
        self.sharding = NamedSharding(self.mesh, PartitionSpec("core"))
        self.in_names, self.out_names = in_names, out_names
        self.out_avals, self.zero_outs = out_avals, zero_outs
        self.jax, self.jnp = jax, jnp

    def stage(self, in_maps):
        """device_put concat inputs + zero outs; returns arg list."""
        jax = self.jax
        args = []
        for i, name in enumerate(self.in_names):
            cat = np.concatenate([np.asarray(m[name]) for m in in_maps], 0)
            args.append(jax.device_put(cat, self.sharding))
        for z in self.zero_outs:
            zz = np.zeros((N_CORES * z.shape[0], *z.shape[1:]), z.dtype)
            args.append(jax.device_put(zz, self.sharding))
        return args

    def __call__(self, args):
        outs = self.fn(*args)
        self.jax.block_until_ready(outs)
        return outs


_CACHE: dict = {}


def _get_exec(key, r1, r2, T, nblk, loadw):
    if key not in _CACHE:
        nc = build_program(r1, r2, T, nblk=nblk, loadw=loadw)
        _CACHE[key] = (nc, _Exec(nc))
    return _CACHE[key]


def run_filter(x: np.ndarray, sos: np.ndarray, T: int = T_FULL,
               nblk: int = 512, loadw: int = 1024, time_reps: int = 0):
    """x: (256, T) float32 -> (y (256, T) float32, times list[s])."""
    import time as _time
    K = T // L
    consts = derive_constants(sos, K)
    key = (sos.astype(np.float32).tobytes(), T, nblk, loadw)
    nc, ex = _get_exec(key, consts["r1"], consts["r2"], T, nblk, loadw)

    shards = x.reshape(N_CORES, NSIG, T)
    base = {k: consts[k] for k in ("hT", "fT", "gT4", "cc", "ss", "rr")}
    base["ident"] = np.eye(128, dtype=np.float32)
    in_maps = [dict(base, x=np.ascontiguousarray(shards[i]))
               for i in range(N_CORES)]
    args = ex.stage(in_maps)
    outs = ex(args)                       # first call compiles + runs
    times = []
    for _ in range(time_reps):
        t0 = _time.perf_counter()
        outs2 = ex(args)
        times.append(_time.perf_counter() - t0)
    oi = ex.out_names.index("y")
    y = np.asarray(outs[oi]).reshape(N_CORES * NSIG, T)
    return y, times


def kernel(x: np.ndarray, sos: np.ndarray) -> np.ndarray:
    x = np.asarray(x, dtype=np.float32)
    sos = np.asarray(sos, dtype=np.float32)
    y, _ = run_filter(x.reshape(B * C, T_FULL), sos)
    return y.reshape(B, C, T_FULL).astype(np.float32)


# revision 54
# speedup vs baseline: 1.0768x; 1.0110x over previous
"""Butterworth 4th-order lowpass (2 cascaded biquads) on 8 TRN2 NeuronCores.

Algorithm: block state-space decomposition of the IIR cascade.
  - Chunk the time axis into L=128 blocks. Within a chunk, the zero-state
    response is a lower-triangular Toeplitz matmul; chunk-boundary states
    follow s_k = M s_{k-1} + f_k with M = A^L, diagonalized into two
    complex modes solved by first-order REAL scans (DVE tensor_tensor_scan)
    via the rotation trick.
  - The output is computed directly in chunk-row-major layout as
       Y[col, t] = (X^T)^T H^T + S^T G''^T
    i.e. matmul with lhsT = time-major X slab (bf16) and rhs = H^T (bf16),
    plus an accumulated state-correction matmul (lhsT = S, rhs = G''^T).
    bf16 matmuls run 1 cycle/row even at 128-wide outputs, and the result
    lands store-ready (no output transpose pass).
  - The state path (F-pass projection, scan) stays f32 for accuracy.
Sharding: 256 independent signals, 32 per core, no cross-core comm.
"""
import numpy as np
from contextlib import ExitStack

import concourse.bass as bass
import concourse.tile as tile
from concourse import bacc, mybir
from concourse.bass_utils import run_bass_kernel_spmd

dt = mybir.dt

B, C, T_FULL = 32, 8, 96000
N_CORES = 8
NSIG = (B * C) // N_CORES      # 32 signals per core
L = 128                        # chunk length
SEG = (8, 8, 8, 8)             # segment sizes; 4*size and 4*offset must be 32-aligned
SEGOFF = (0, 8, 16, 24)        # signal offset of each segment


# ---------------------------------------------------------------- host math
def derive_constants(sos: np.ndarray, K: int):
    """Constant matrices for the block SSM, float64. K = chunks per signal."""
    sos = sos.astype(np.float64)
    (b0, b1, b2, a1, a2), (B0, B1, B2, A1, A2) = [
        (s[0] / s[3], s[1] / s[3], s[2] / s[3], s[4] / s[3], s[5] / s[3])
        for s in sos
    ]
    c1, c2 = b1 - b0 * a1, b2 - b0 * a2
    A = np.array([
        [-a1, -a2, 0.0, 0.0],
        [1.0, 0.0, 0.0, 0.0],
        [c1, c2, -A1, -A2],
        [0.0, 0.0, 1.0, 0.0],
    ])
    Bv = np.array([1.0, 0.0, b0, 0.0])
    Cv = np.array([B0 * c1, B0 * c2, B1 - B0 * A1, B2 - B0 * A2])
    D = B0 * b0

    h = np.zeros(L)
    h[0] = D
    s = Bv.copy()
    for t in range(1, L):
        h[t] = Cv @ s
        s = A @ s
    H = np.zeros((L, L))
    for j in range(L):
        H[j:, j] = h[: L - j]

    Fm = np.zeros((4, L))
    Ap = np.eye(4)
    for j in range(L - 1, -1, -1):
        Fm[:, j] = Ap @ Bv
        Ap = A @ Ap
    G = np.zeros((L, 4))
    Ap = np.eye(4)
    for t in range(L):
        G[t, :] = Cv @ Ap
        Ap = A @ Ap

    M = np.linalg.matrix_power(A, L)
    lam, V = np.linalg.eig(M)
    idx = [i for i in range(4) if lam[i].imag > 0]
    assert len(idx) == 2, lam
    lam2, V2 = lam[idx], V[:, idx]
    Vinv2 = np.linalg.inv(V)[idx, :]

    Fmod = Vinv2 @ Fm                      # (2, L) complex
    Fp = np.stack([Fmod[0].real, Fmod[0].imag, Fmod[1].real, Fmod[1].imag])
    GV = G @ V2                            # (L, 2) complex
    Gpp = np.stack([2 * GV[:, 0].real, -2 * GV[:, 0].imag,
                    2 * GV[:, 1].real, -2 * GV[:, 1].imag], axis=1)

    r, th = np.abs(lam2), np.angle(lam2)
    k = np.arange(K)
    # per segment: (mode, signal) block layout, rows 4*off + a*nsig + n
    CCh = np.zeros((4 * NSIG, K), dtype=np.float64)
    SSh = np.zeros((4 * NSIG, K), dtype=np.float64)
    RRh = np.zeros((4 * NSIG, 1), dtype=np.float64)
    for off, ns in zip(SEGOFF, SEG):
        r0 = 4 * off
        for a in range(4):
            e = a // 2
            CCh[r0 + a * ns:r0 + (a + 1) * ns, :] = np.cos(th[e] * k)[None, :]
            SSh[r0 + a * ns:r0 + (a + 1) * ns, :] = \
                (1.0 if a % 2 == 0 else -1.0) * np.sin(th[e] * k)[None, :]
            RRh[r0 + a * ns:r0 + (a + 1) * ns, 0] = r[e]

    f32 = np.float32
    return dict(
        hT=np.ascontiguousarray(H.T, dtype=f32),       # (L, L)  = H^T, rhs
        fT=np.ascontiguousarray(Fp.T, dtype=f32),      # (L, 4)  lhsT F-pass
        gT4=np.ascontiguousarray(Gpp.T, dtype=f32),    # (4, L)  = G''^T, rhs
        cc=np.ascontiguousarray(CCh, dtype=f32),       # (4*NSIG, K)
        ss=np.ascontiguousarray(SSh, dtype=f32),       # (4*HS, K)
        rr=np.ascontiguousarray(np.broadcast_to(RRh, (4 * NSIG, K)),
                                dtype=f32),
        r1=float(r[0]), r2=float(r[1]),
    )


# ---------------------------------------------------------------- program
def build_program(r1: float, r2: float, T: int, nblk: int = 512,
                  loadw: int = 1024, order: str = "A"):
    """Build + compile the per-core Bass program."""
    K = T // L                  # chunks per signal
    COLS = NSIG * K             # total chunk-columns
    assert T % L == 0

    nc = bacc.Bacc("TRN2", target_bir_lowering=False, debug=False,
                   num_devices=N_CORES)
    x_d = nc.dram_tensor("x", [NSIG, T], dt.float32, kind="ExternalInput").ap()
    y_d = nc.dram_tensor("y", [NSIG, T], dt.float32, kind="ExternalOutput").ap()
    ident_d = nc.dram_tensor("ident", [128, 128], dt.float32, kind="ExternalInput").ap()
    hT_d = nc.dram_tensor("hT", [L, L], dt.float32, kind="ExternalInput").ap()
    fT_d = nc.dram_tensor("fT", [L, 4], dt.float32, kind="ExternalInput").ap()
    gT4_d = nc.dram_tensor("gT4", [4, L], dt.float32, kind="ExternalInput").ap()
    cc_d = nc.dram_tensor("cc", [4 * NSIG, K], dt.float32, kind="ExternalInput").ap()
    ss_d = nc.dram_tensor("ss", [4 * NSIG, K], dt.float32, kind="ExternalInput").ap()
    rr_d = nc.dram_tensor("rr", [4 * NSIG, K], dt.float32, kind="ExternalInput").ap()

    x_flat = x_d.rearrange("a b -> (a b)")
    y_flat = y_d.rearrange("a b -> (a b)")

    with tile.TileContext(nc) as tc, ExitStack() as ctx:
        consts = ctx.enter_context(tc.tile_pool(name="consts", bufs=1))
        scanp = ctx.enter_context(tc.tile_pool(name="scan", bufs=1))
        xtp = ctx.enter_context(tc.tile_pool(name="xt", bufs=1))
        ldp = ctx.enter_context(tc.tile_pool(name="ld", bufs=3))
        fsbp = ctx.enter_context(tc.tile_pool(name="fsb", bufs=5))
        sblkp = ctx.enter_context(tc.tile_pool(name="sblk", bufs=1))
        dramp = ctx.enter_context(tc.tile_pool(name="dram", bufs=1, space="DRAM"))
        youtp = ctx.enter_context(tc.tile_pool(name="yout", bufs=4))
        ps_t = ctx.enter_context(tc.tile_pool(name="ps_t", bufs=4, space="PSUM"))
        ps_f = ctx.enter_context(tc.tile_pool(name="ps_f", bufs=2, space="PSUM"))
        ps_y = ctx.enter_context(tc.tile_pool(name="ps_y", bufs=2, space="PSUM"))

        # ---- constants: f32 originals + bf16/f32r device copies
        ident = consts.tile([128, 128], dt.float32)
        nc.scalar.dma_start(ident[:], ident_d[:])
        hT = consts.tile([L, L], dt.float32)
        nc.scalar.dma_start(hT[:], hT_d[:])
        hTb = consts.tile([L, L], dt.float16)
        nc.scalar.copy(hTb[:], hT[:])
        fT = consts.tile([L, 4], dt.float32)
        nc.scalar.dma_start(fT[:], fT_d[:])
        fTb = consts.tile([L, 4], dt.float16)
        nc.scalar.copy(fTb[:], fT[:])
        gT4 = consts.tile([4, L], dt.float32)
        nc.scalar.dma_start(gT4[:], gT4_d[:])
        gT4b = consts.tile([4, L], dt.float16)
        nc.scalar.copy(gT4b[:], gT4[:])
        cc_sz, ss_sz, rr_sz = {}, {}, {}
        for off2, ns2 in zip(SEGOFF, SEG):
            if ns2 in cc_sz:
                continue            # same-size segments share identical blocks
            r0, r1_ = 4 * off2, 4 * off2 + 4 * ns2
            cch = consts.tile([4 * ns2, K], dt.float32, name=f"cc{ns2}")
            nc.scalar.dma_start(cch[:], cc_d[r0:r1_, :])
            ssh = consts.tile([4 * ns2, K], dt.float32, name=f"ss{ns2}")
            nc.scalar.dma_start(ssh[:], ss_d[r0:r1_, :])
            rrh = consts.tile([4 * ns2, K], dt.float32, name=f"rr{ns2}")
            nc.scalar.dma_start(rrh[:], rr_d[r0:r1_, :])
            cc_sz[ns2], ss_sz[ns2], rr_sz[ns2] = cch, ssh, rrh

        Xtb = xtp.tile([128, COLS], dt.float16)    # X^T, time-major, bf16

        tS_tiles = {}
        gb32_tiles = {}
        sblk_tiles = {}
        g_tiles = {}
        eng_ns = {"act": 0.0, "dve": 0.0}

        def bal_copy(out_ap, in_ap, width):
            act_cost = width * 0.833 + 143.0
            dve_cost = width * 1.042 + 125.0
            if eng_ns["act"] + act_cost <= eng_ns["dve"] + dve_cost:
                eng_ns["act"] += act_cost
                nc.scalar.copy(out_ap, in_ap)
            else:
                eng_ns["dve"] += dve_cost
                nc.vector.tensor_copy(out_ap, in_ap)

        def act_copy(out_ap, in_ap, width):
            eng_ns["act"] += width * 0.833 + 143.0
            nc.scalar.copy(out_ap, in_ap)

        def seg_geo(h):
            off, ns = SEGOFF[h], SEG[h]
            return off * K, (off + ns) * K, ns, 4 * off

        def in_phase(h):
            hc0, hc1, ns, crow = seg_geo(h)
            gb32 = dramp.tile([4, ns * K], dt.float32, tag=f"gb32_{h}")
            gb32_tiles[h] = gb32
            FW = 4 * nblk

            state = {"psf": None, "pf_base": 0, "pf_off": 0,
                     "fsb": None, "fsb_base": 0, "fsb_off": 0}

            def f_group(c0, gw):
                """F matmul for cols [c0, c0+gw) + staged copy/bounce."""
                if state["psf"] is not None and state["pf_off"] + gw > nblk:
                    flush_psf()
                if state["psf"] is None:
                    state["psf"] = ps_f.tile([4, nblk], dt.float32,
                                             tag="psf", name="psf")
                    state["pf_base"], state["pf_off"] = c0, 0
                o = state["pf_off"]
                nc.tensor.matmul(state["psf"][:, o:o + gw],
                                 fTb[:], Xtb[:, c0:c0 + gw],
                                 start=True, stop=True)
                state["pf_off"] = o + gw

            def flush_psf():
                if state["psf"] is None:
                    return
                pw = state["pf_off"]
                pb = state["pf_base"]
                if state["fsb"] is not None and state["fsb_off"] + pw > FW:
                    flush_fsb()
                if state["fsb"] is None:
                    state["fsb"] = fsbp.tile([4, FW], dt.float32,
                                             tag="fsb", name="fsb")
                    state["fsb_base"], state["fsb_off"] = pb, 0
                fo = state["fsb_off"]
                bal_copy(state["fsb"][:, fo:fo + pw], state["psf"][:, 0:pw],
                         pw)
                state["fsb_off"] = fo + pw
                state["psf"] = None

            def flush_fsb():
                if state["fsb"] is None:
                    return
                fb, fw = state["fsb_base"], state["fsb_off"]
                nc.gpsimd.dma_start(gb32[:, fb - hc0:fb - hc0 + fw],
                                    state["fsb"][:, 0:fw])
                state["fsb"] = None

            # loads (SP queue), chunk-major tiles
            ld_tiles = []
            r0 = hc0
            while r0 < hc1:
                w = min(loadw, hc1 - r0)
                fullw = (w // 128) * 128
                if fullw:
                    t_in = ldp.tile([128, loadw], dt.float32, tag="ld")
                    view = x_flat[r0 * 128:(r0 + fullw) * 128].rearrange(
                        "(q p t) -> p q t", p=128, t=128)
                    dst = t_in[:, 0:fullw].rearrange("p (q t) -> p q t", t=128)
                    nc.sync.dma_start(dst, view)
                    ld_tiles.append((t_in, r0, fullw))
                if w > fullw:
                    rem = w - fullw
                    t_in2 = ldp.tile([128, 128], dt.float32, tag="ldp")
                    view = x_flat[(r0 + fullw) * 128:(r0 + w) * 128].rearrange(
                        "(p t) -> p t", p=rem, t=128)
                    nc.sync.dma_start(t_in2[0:rem, :], view)
                    ld_tiles.append((t_in2, r0 + fullw, rem))
                r0 += w

            # PE transposes -> psum -> copy -> Xtb (fp16), F per group
            for t_in, col0, width in ld_tiles:
                if width >= 128:
                    for g0 in range(0, width, 512):
                        gw = min(512, width - g0)
                        pst = ps_t.tile([128, 512], dt.float32, tag="pst")
                        for q in range(gw // 128):
                            nc.tensor.transpose(
                                pst[:, q * 128:(q + 1) * 128],
                                t_in[:, g0 + q * 128:g0 + (q + 1) * 128],
                                ident[:])
                        bal_copy(Xtb[:, col0 + g0:col0 + g0 + gw],
                                 pst[:, 0:gw], gw)
                        f_group(col0 + g0, gw)
                else:
                    pst2 = ps_t.tile([128, 512], dt.float32, tag="pst")
                    nc.tensor.transpose(pst2[0:128, 0:width],
                                        t_in[0:width, 0:128],
                                        ident[0:width, 0:width])
                    bal_copy(Xtb[:, col0:col0 + width],
                             pst2[:, 0:width], width)
                    f_group(col0, width)
            flush_psf()
            flush_fsb()

        def scan_dma(h):
            hc0, hc1, ns, crow = seg_geo(h)
            gb32 = gb32_tiles[h]
            gbv = gb32[:].rearrange("a c -> (a c)").rearrange(
                "(r k) -> r k", k=K)
            # g_t[a*ns+n, k] = gb32[a, n*K+k]   (ACT queue)
            gtag = "A" if h % 2 == 0 else "B"
            g_t = scanp.tile([4 * max(SEG), K], dt.float32,
                             tag=f"g_t{gtag}", name="g_t")
            nc.gpsimd.dma_start(g_t[0:4 * ns, :], gbv)
            # gswap = mode-pair swap of g_t
            gswap = scanp.tile([4 * max(SEG), K], dt.float32,
                               tag=f"gswap{gtag}", name="gswap")
            for (d0, s0) in ((0, ns), (ns, 0), (2 * ns, 3 * ns),
                             (3 * ns, 2 * ns)):
                nc.gpsimd.dma_start(gswap[d0:d0 + ns, :],
                                    gbv[s0:s0 + ns, :])
            g_tiles[h] = (g_t, gswap)

        def scan_ops(h):
            hc0, hc1, ns, crow = seg_geo(h)
            g_t, gswap = g_tiles[h]
            ns_ = SEG[h]
            ccs = cc_sz[ns_][:]
            sss = ss_sz[ns_][:]
            rrs = rr_sz[ns_][:]
            # rotation trick:  gt_tw = cc*g + ss*gswap ;  m = scan(r, gt_tw)
            # tS[:, k+1] = cc*m - ss*swap(m)
            gt_tw = scanp.tile([4 * ns, K], dt.float32, tag="gt_tw",
                               name="gt_tw")
            tmp1 = scanp.tile([4 * ns, K], dt.float32, tag="scr1",
                              name="tmp1")
            tmp2 = scanp.tile([4 * ns, K], dt.float32, tag="scr2",
                              name="tmp2")
            nc.vector.tensor_mul(tmp1[:], ccs, g_t[0:4 * ns, :])
            nc.vector.tensor_mul(tmp2[:], sss, gswap[0:4 * ns, :])
            nc.vector.tensor_add(gt_tw[:], tmp1[:], tmp2[:])
            m_t = scanp.tile([4 * ns, K], dt.float32, tag="m_t", name="m_t")
            nc.vector.tensor_tensor_scan(
                m_t[:], rrs, gt_tw[:], 0.0,
                mybir.AluOpType.mult, mybir.AluOpType.add)
            mswap = scanp.tile([4 * ns, K], dt.float32, tag="mswap",
                               name="mswap")
            for (d0, s0) in ((0, ns), (ns, 0), (2 * ns, 3 * ns),
                             (3 * ns, 2 * ns)):
                nc.gpsimd.dma_start(mswap[d0:d0 + ns, :],
                                    m_t[s0:s0 + ns, :])
            tS = scanp.tile([4 * ns, K + 1], dt.float16, tag=f"tS{h}",
                            name="tS")
            tS_tiles[h] = tS
            nc.vector.memset(tS[:, 0:1], 0.0)
            t1b = scanp.tile([4 * ns, K], dt.float32, tag="scr1", name="t1b")
            t2b = scanp.tile([4 * ns, K], dt.float32, tag="scr2", name="t2b")
            nc.vector.tensor_mul(t1b[:], ccs, m_t[:])
            nc.vector.tensor_mul(t2b[:], sss, mswap[:])
            nc.vector.tensor_sub(tS[:, 1:K + 1], t1b[:], t2b[:])
            eng_ns["dve"] += 7 * (K * 1.042 + 125.0)
            gtag2 = "A" if h % 2 == 0 else "B"
            # stage S into G-pass lhsT layout: sblk[a, n*K+k] = tS[a*ns+n, k]
            sblk = sblkp.tile([4, max(SEG) * K], dt.float16,
                              tag=f"sblk{gtag2}", name="sblk")
            sblk_tiles[h] = sblk
            for n in range(ns):
                nc.sync.dma_start(sblk[:, n * K:(n + 1) * K],
                                  tS[n::ns, 0:K])

        def out_phase(h):
            hc0, hc1, ns, crow = seg_geo(h)
            sblk = sblk_tiles[h]
            storew = 1024
            yout, yo_base = None, 0
            c0 = hc0
            while c0 < hc1:
                c1 = min(c0 + nblk, hc1)
                n = c1 - c0
                if yout is None:
                    yout = youtp.tile([128, storew], dt.float32, tag="yout")
                    yo_base = c0
                psy = ps_y.tile([128, 512], dt.float32, tag="psy")
                # per 128-col slab: Y[col, t] = Xslab^T H^T + Sslab^T G''^T
                q = 0
                while q * 128 < n:
                    w = min(128, n - q * 128)
                    sc0 = c0 + q * 128
                    nc.tensor.matmul(psy[0:w, q * 128:q * 128 + 128],
                                     Xtb[:, sc0:sc0 + w], hTb[:],
                                     start=True, stop=False)
                    nc.tensor.matmul(psy[0:w, q * 128:q * 128 + 128],
                                     sblk[:, sc0 - hc0:sc0 - hc0 + w],
                                     gT4b[:],
                                     start=False, stop=True)
                    q += 1
                fullq = n // 128
                off0 = c0 - yo_base
                if fullq:
                    bal_copy(yout[:, off0:off0 + fullq * 128],
                             psy[:, 0:fullq * 128], fullq * 128)
                if n > fullq * 128:
                    w = n - fullq * 128
                    bal_copy(
                        yout[0:w, off0 + fullq * 128:off0 + fullq * 128 + 128],
                        psy[0:w, fullq * 128:fullq * 128 + 128], 128)

                if c1 - yo_base >= storew or c1 == hc1:
                    wq = c1 - yo_base
                    fullw = (wq // 128) * 128
                    if fullw:
                        view = y_flat[yo_base * 128:(yo_base + fullw) * 128] \
                            .rearrange("(qq p t) -> p qq t", p=128, t=128)
                        srcv = yout[:, 0:fullw].rearrange(
                            "p (qq t) -> p qq t", t=128)
                        nc.scalar.dma_start(view, srcv)
                    if wq > fullw:
                        rem = wq - fullw
                        view = y_flat[(yo_base + fullw) * 128:
                                      (yo_base + wq) * 128].rearrange(
                            "(p t) -> p t", p=rem, t=128)
                        nc.scalar.dma_start(view, yout[0:rem,
                                                       fullw:fullw + 128])
                    yout = None
                c0 = c1

        # software pipeline: per-engine FIFO order must match dependency
        # readiness (in-order SEQs); g-loads precede older stores on ACT
        in_phase(0)
        scan_dma(0)
        in_phase(1)
        scan_ops(0)
        scan_dma(1)
        in_phase(2)
        scan_ops(1)
        out_phase(0)
        scan_dma(2)
        in_phase(3)
        scan_ops(2)
        out_phase(1)
        scan_dma(3)
        scan_ops(3)
        out_phase(2)
        out_phase(3)
    nc.compile()
    return nc


class _Exec:
    """Cached PJRT executable for one built program (8-core shard_map)."""

    def __init__(self, nc):
        import jax
        import jax.numpy as jnp
        from jax.sharding import Mesh, PartitionSpec, NamedSharding
        try:
            from jax.experimental.shard_map import shard_map
        except ImportError:
            from jax import shard_map
        from concourse import bass2jax
        from concourse.bass2jax import _bass_exec_p, partition_id_tensor

        bass2jax.install_neuronx_cc_hook()
        assert nc.dbg_addr is None
        pname = nc.partition_id_tensor.name if nc.partition_id_tensor else None
        in_names, out_names, out_avals, zero_outs = [], [], [], []
        for alloc in nc.m.functions[0].allocations:
            if not isinstance(alloc, mybir.MemoryLocationSet):
                continue
            name = alloc.memorylocations[0].name
            if alloc.kind == "ExternalInput":
                if name != pname:
                    in_names.append(name)
            elif alloc.kind == "ExternalOutput":
                shape = tuple(alloc.tensor_shape)
                dtype = mybir.dt.np(alloc.dtype)
                out_names.append(name)
                out_avals.append(jax.core.ShapedArray(shape, dtype))
                zero_outs.append(np.zeros(shape, dtype))
        n_params = len(in_names)
        all_in = in_names + out_names + ([pname] if pname else [])

        def _body(*args):
            operands = list(args)
            if pname is not None:
                operands.append(partition_id_tensor())
            return tuple(_bass_exec_p.bind(
                *operands,
                out_avals=tuple(out_avals),
                in_names=tuple(all_in),
                out_names=tuple(out_names),
                lowering_input_output_aliases=(),
                sim_require_finite=True,
                sim_require_nnan=True,
                nc=nc,
            ))

        devices = jax.devices()[:N_CORES]
        self.mesh = Mesh(np.asarray(devices), ("core",))
        nin = n_params + len(zero_outs)
        self.fn = jax.jit(shard_map(
            _body, mesh=self.mesh,
            in_specs=(PartitionSpec("core"),) * nin,
            out_specs=(PartitionSpec("core"),) * len(out_names),
            check_rep=False))
        self.sharding = NamedSharding(self.mesh, PartitionSpec("core"))
        self.in_names, self.out_names = in_names, out_names
        self.out_avals, self.zero_outs = out_avals, zero_outs
        self.jax, self.jnp = jax, jnp

    def stage(self, in_maps):
        """device_put concat inputs + zero outs; returns arg list."""
        jax = self.jax
        args = []
        for i, name in enumerate(self.in_names):
            cat = np.concatenate([np.asarray(m[name]) for m in in_maps], 0)
            args.append(jax.device_put(cat, self.sharding))
        for z in self.zero_outs:
            zz = np.zeros((N_CORES * z.shape[0], *z.shape[1:]), z.dtype)
            args.append(jax.device_put(zz, self.sharding))
        return args

    def __call__(self, args):
        outs = self.fn(*args)
        self.jax.block_until_ready(outs)
        return outs


_CACHE: dict = {}


def _get_exec(key, r1, r2, T, nblk, loadw):
    if key not in _CACHE:
        nc = build_program(r1, r2, T, nblk=nblk, loadw=loadw)
        _CACHE[key] = (nc, _Exec(nc))
    return _CACHE[key]


def run_filter(x: np.ndarray, sos: np.ndarray, T: int = T_FULL,
               nblk: int = 512, loadw: int = 1024, time_reps: int = 0):
    """x: (256, T) float32 -> (y (256, T) float32, times list[s])."""
    import time as _time
    K = T // L
    consts = derive_constants(sos, K)
    key = (sos.astype(np.float32).tobytes(), T, nblk, loadw)
    nc, ex = _get_exec(key, consts["r1"], consts["r2"], T, nblk, loadw)

    shards = x.reshape(N_CORES, NSIG, T)
    base = {k: consts[k] for k in ("hT", "fT", "gT4", "cc", "ss", "rr")}
    base["ident"] = np.eye(128, dtype=np.float32)
    in_maps = [dict(base, x=np.ascontiguousarray(shards[i]))
               for i in range(N_CORES)]
    args = ex.stage(in_maps)
    outs = ex(args)                       # first call compiles + runs
    times = []
    for _ in range(time_reps):
        t0 = _time.perf_counter()
        outs2 = ex(args)
        times.append(_time.perf_counter() - t0)
    oi = ex.out_names.index("y")
    y = np.asarray(outs[oi]).reshape(N_CORES * NSIG, T)
    return y, times


def kernel(x: np.ndarray, sos: np.ndarray) -> np.ndarray:
    x = np.asarray(x, dtype=np.float32)
    sos = np.asarray(sos, dtype=np.float32)
    y, _ = run_filter(x.reshape(B * C, T_FULL), sos)
    return y.reshape(B, C, T_FULL).astype(np.float32)
